# revision 1
# baseline (speedup 1.0000x reference)
"""Trainium2 Bass kernel for Disk descriptor mutual-NN matching (retrieval_knn).

Strategy (8 NeuronCores, shard descriptors1 columns M across cores):
  - Each core c holds full d0 [256, 8192] and its d1 shard [256, 1024].
  - Forward:  S_c = d0.T @ d1_c  (fp32 PE matmul) -> per-row (N) top-8 values
    + indices over the core's 1024 local columns via the DVE top-8
    instruction (InstMax / InstMaxIndex).  Host merges per-core top-2s.
  - Backward: instead of recomputing S^T, PE-transposes the forward S tiles
    (exact bit-preserving data movement) and reduces columns via staged
    InstMax top-8 merges.  Only column top-2 VALUES are needed: the mutual
    check `bck_nn[fwd_nn[i]] == i` is replaced by the exactly-equivalent
    value test `S[i,j] == colmax[j]` (ties in either formulation force the
    backward ratio test to fail identically, so outputs match bit-for-bit).
  - Host applies the exact reference arithmetic in float32 (sqrt transform,
    division-based ratio test, tie semantics).
"""

import sys

if "/opt/trn_rl_repo" not in sys.path:
    sys.path.insert(0, "/opt/trn_rl_repo")

import numpy as np

N_KPTS = 8192
M_KPTS = 8192
F_DIM = 256
N_CORES = 8
M_SHARD = M_KPTS // N_CORES  # 1024

SQRT_2 = np.float32(1.414213)
CLIP_LO = np.float32(1e-6)
ONE = np.float32(1.0)

GROUP = 4  # fwd row-chunks per transpose group

# fp32: native fp32 matmuls (4 cyc/row).  f16x3: split each f32 input into
# f16 high part + scaled f16 residual; S = h0*h1 + h0s*l1s + l0s*h1s
# accumulated in one PSUM group (3 f16 matmuls at 1 cyc/row each = 3/4 the
# PE time).  All products are exact or quantized below fp32 accumulation
# noise, so ranking quality matches native fp32.
DTYPE_MODE = "f16x3"


def _split_f16(a32):
    """f32 -> (h, h/32, 32*(a-h)) as float16, with f16-subnormal highs
    flushed into the residual so no information rides on f16 subnormals."""
    h = a32.astype(np.float16)
    h[np.abs(a32) < 6.104e-5] = np.float16(0)
    l = a32 - h.astype(np.float32)
    h_s = (h.astype(np.float32) / 32.0).astype(np.float16)
    l_s = (l * 32.0).astype(np.float16)
    return h, h_s, l_s


# --------------------------------------------------------------------------
# Device kernel builder
# --------------------------------------------------------------------------

def build_kernel(n_rows=N_KPTS, m_shard=M_SHARD, f_dim=F_DIM, repeat=1,
                 dtype_mode=DTYPE_MODE):
    """Build the per-core SPMD Bass program.

    Inputs (per core):
      d0: [kf, 128, n_rows] f32   (descriptors0, K-chunked)
      d1: [kf, 128, m_shard] f32  (this core's descriptors1 shard)
    Outputs (per core):
      fwd_val [128, n_chunks*8] f32, fwd_idx [128, n_chunks*8] u32
      bwd_val [128, m_chunks*8] f32   (column top-8 values, rows = local col)
    """
    import concourse.bacc as bacc
    import concourse.mybir as mybir
    import concourse.tile as tile
    from concourse.masks import make_identity

    kf = f_dim // 128
    n_chunks = n_rows // 128          # forward row chunks
    m_tiles = max(1, m_shard // 512)  # 512-wide column tiles per fwd chunk
    mw = min(512, m_shard)
    m_chunks = m_shard // 128         # backward column chunks
    assert n_chunks % GROUP == 0 and m_shard % 128 == 0 and f_dim % 128 == 0
    n_groups = n_chunks // GROUP
    pair = 2 if n_groups % 2 == 0 else 1    # transpose groups per staging
    n_pairs = n_groups // pair
    n_stages = n_pairs

    nc = bacc.Bacc("TRN2", target_bir_lowering=False, debug=False,
                   num_devices=1)

    if dtype_mode == "fp32":
        in_names = ["d0", "d1"]
        in_dt = mybir.dt.float32
    else:
        in_names = ["d0h", "d0hs", "d0ls", "d1h", "d1hs", "d1ls"]
        in_dt = mybir.dt.float16
    in_dram = {}
    for nm in in_names:
        nw = n_rows if nm.startswith("d0") else m_shard
        in_dram[nm] = nc.dram_tensor(nm, [kf, 128, nw], in_dt,
                                     kind="ExternalInput")
    fwd_val = nc.dram_tensor("fwd_val", [128, n_chunks * 8], mybir.dt.float32,
                             kind="ExternalOutput")
    fwd_idx = nc.dram_tensor("fwd_idx", [128, n_chunks * 8], mybir.dt.uint32,
                             kind="ExternalOutput")
    bwd_val = nc.dram_tensor("bwd_val", [128, m_chunks * 8], mybir.dt.float32,
                             kind="ExternalOutput")

    with tile.TileContext(nc) as tc:
        with tc.tile_pool(name="persist", bufs=1) as persist, \
             tc.tile_pool(name="schunk", bufs=pair * GROUP + 2) as schunk_pool, \
             tc.tile_pool(name="stg", bufs=3) as stg_pool, \
             tc.tile_pool(name="outs", bufs=1) as outs_pool, \
             tc.tile_pool(name="psf", bufs=2, space="PSUM") as psf, \
             tc.tile_pool(name="ptp", bufs=2, space="PSUM") as ptp:

            # resident inputs; d0 loads split along n so early fwd units
            # unblock before the full load completes
            in_sb = {}
            for nm in in_names:
                nw = n_rows if nm.startswith("d0") else m_shard
                in_sb[nm] = [persist.tile([128, nw], in_dt,
                                          name=f"{nm}sb{k}", tag=f"{nm}sb{k}")
                             for k in range(kf)]
            # interleave d1 loads with the first d0 piece so the critical
            # first-unit inputs land on distinct DMA queues immediately
            n_split = 8 if n_rows % 1024 == 0 else 1
            d0n = [nm for nm in in_names if nm.startswith("d0")]
            d1n = [nm for nm in in_names if nm.startswith("d1")]
            first = []
            for k in range(kf):
                for i in range(max(len(d0n), len(d1n))):
                    if i < len(d1n):
                        first.append((d1n[i], k, None))
                    if i < len(d0n):
                        first.append((d0n[i], k, 0))
            for nm, k, p in first:
                if p is None:
                    nc.sync.dma_start(in_sb[nm][k][:], in_dram[nm][k])
                else:
                    sl = slice(0, n_rows // n_split)
                    nc.sync.dma_start(in_sb[nm][k][:, sl],
                                      in_dram[nm][k][:, sl])
            for p in range(1, n_split):
                sl = slice(p * n_rows // n_split, (p + 1) * n_rows // n_split)
                for nm in d0n:
                    for k in range(kf):
                        nc.sync.dma_start(in_sb[nm][k][:, sl],
                                          in_dram[nm][k][:, sl])
            if dtype_mode == "fp32":
                # (lhsT source, rhs source) per accumulation term
                terms = [("d0", "d1")]
            else:
                terms = [("d0h", "d1h"), ("d0hs", "d1ls"), ("d0ls", "d1hs")]


            ident = persist.tile([128, 128], mybir.dt.float32, name="ident")
            make_identity(nc, ident[:])
            # warm-up matmul: starts the PE p-state ramp clock while input
            # DMAs are still streaming (identity needs no DMA)
            warm = psf.tile([128, 8], mybir.dt.float32, tag="pf", name="warm",
                            padded_shape=[128, m_tiles * mw])
            nc.tensor.matmul(warm[:], ident[:], ident[:, :8],
                             start=True, stop=True)

            fv_sb = outs_pool.tile([128, n_chunks * 8], mybir.dt.float32)
            fi_sb = outs_pool.tile([128, n_chunks * 8], mybir.dt.uint32)
            bv_sb = outs_pool.tile([128, m_chunks * 8], mybir.dt.float32)
            # per-mm candidate buffers (top-8 of each staging window)
            cand = [outs_pool.tile([128, max(8, n_stages * 8)],
                                   mybir.dt.float32, name=f"cand{mm}",
                                   tag=f"cand{mm}")
                    for mm in range(m_chunks)]

            for _rep in range(repeat):
                def fwd_unit(n):
                    s_chunk = schunk_pool.tile([128, m_shard],
                                               mybir.dt.float32, tag="schunk")
                    # one PSUM tile spanning m_tiles banks; each matmul
                    # writes within a single bank; one wide ACT copy drains
                    pf = psf.tile([128, m_tiles * mw], mybir.dt.float32,
                                  tag="pf", name="pf")
                    n_acc = kf * len(terms)
                    for k in range(kf):
                        for ti, (lnm, rnm) in enumerate(terms):
                            # weight (lhsT) loaded once, reused across m
                            for m in range(m_tiles):
                                acc = k * len(terms) + ti
                                nc.tensor.matmul(
                                    pf[:, m * mw:(m + 1) * mw],
                                    in_sb[lnm][k][:, n * 128:(n + 1) * 128],
                                    in_sb[rnm][k][:, m * mw:(m + 1) * mw],
                                    start=(acc == 0), stop=(acc == n_acc - 1))
                    nc.scalar.copy(s_chunk[:], pf[:])
                    nc.vector.max(out=fv_sb[:, n * 8:(n + 1) * 8],
                                  in_=s_chunk[:])
                    nc.vector.max_index(out=fi_sb[:, n * 8:(n + 1) * 8],
                                        in_max=fv_sb[:, n * 8:(n + 1) * 8],
                                        in_values=s_chunk[:])
                    return s_chunk

                wpp = pair * GROUP * 128  # rows covered per group-pair
                for gp in range(n_pairs):
                    chunks = [fwd_unit(gp * pair * GROUP + j)
                              for j in range(pair * GROUP)]
                    for mm in range(m_chunks):
                        pt = ptp.tile([128, wpp], mybir.dt.float32, tag="pt")
                        for j in range(pair * GROUP):
                            nc.tensor.transpose(
                                pt[:, j * 128:(j + 1) * 128],
                                chunks[j][:, mm * 128:(mm + 1) * 128],
                                ident[:])
                        stg = stg_pool.tile([128, wpp], mybir.dt.float32,
                                            name="stg", tag="stg")
                        nc.scalar.copy(stg[:], pt[:])
                        nc.vector.max(out=cand[mm][:, gp * 8:(gp + 1) * 8],
                                      in_=stg[:])
                for mm in range(m_chunks):
                    if n_pairs > 1:
                        nc.vector.max(out=bv_sb[:, mm * 8:(mm + 1) * 8],
                                      in_=cand[mm][:])
                    else:
                        nc.vector.tensor_copy(bv_sb[:, mm * 8:(mm + 1) * 8],
                                              cand[mm][:, :8])

            # stream forward outputs out as they complete (shorter tail)
            ow = n_chunks * 8 // max(1, min(4, n_pairs))
            for p in range(n_chunks * 8 // ow):
                sl = slice(p * ow, (p + 1) * ow)
                nc.sync.dma_start(fwd_val[:, sl], fv_sb[:, sl])
                nc.sync.dma_start(fwd_idx[:, sl], fi_sb[:, sl])
            nc.sync.dma_start(bwd_val[:], bv_sb[:])

    nc.compile()
    return nc


_KERNEL_CACHE = {}


def get_kernel(repeat=1, dtype_mode=DTYPE_MODE):
    key = (repeat, dtype_mode)
    if key not in _KERNEL_CACHE:
        _KERNEL_CACHE[key] = build_kernel(repeat=repeat,
                                          dtype_mode=dtype_mode)
    return _KERNEL_CACHE[key]


# --------------------------------------------------------------------------
# Host side
# --------------------------------------------------------------------------

def _decode_top8(arr, chunks):
    """[128, chunks*8] -> [chunks*128, 8] with row r = chunk*128 + partition."""
    return arr.reshape(128, chunks, 8).transpose(1, 0, 2).reshape(chunks * 128, 8)


def run_device(descriptors0, descriptors1, repeat=1, dtype_mode=DTYPE_MODE):
    """Run the SPMD kernel on 8 cores. Returns per-core raw outputs."""
    from concourse.bass_utils import run_bass_kernel_spmd

    nc = get_kernel(repeat, dtype_mode)
    d0 = np.ascontiguousarray(descriptors0[0]).astype(np.float32, copy=False)
    d1 = np.ascontiguousarray(descriptors1[0]).astype(np.float32, copy=False)
    kf = F_DIM // 128

    def shard(a, c):
        return np.ascontiguousarray(
            a[:, c * M_SHARD:(c + 1) * M_SHARD]).reshape(kf, 128, M_SHARD)

    if dtype_mode == "fp32":
        d0r = d0.reshape(kf, 128, N_KPTS)
        in_maps = [{"d0": d0r, "d1": shard(d1, c)} for c in range(N_CORES)]
    else:
        h0, h0s, l0s = _split_f16(d0)
        h1, h1s, l1s = _split_f16(d1)
        d0m = {"d0h": h0.reshape(kf, 128, N_KPTS),
               "d0hs": h0s.reshape(kf, 128, N_KPTS),
               "d0ls": l0s.reshape(kf, 128, N_KPTS)}
        in_maps = [dict(d0m, d1h=shard(h1, c), d1hs=shard(h1s, c),
                        d1ls=shard(l1s, c)) for c in range(N_CORES)]
    last_err = None
    for _attempt in range(3):
        try:
            res = run_bass_kernel_spmd(nc, in_maps, list(range(N_CORES)))
            return res.results
        except Exception as e:  # rare transient device-unrecoverable flakes
            last_err = e
    raise last_err


def postprocess(results):
    """Merge per-core device outputs into the reference's 4 output arrays."""
    n = N_KPTS
    n_chunks = n // 128
    m_chunks = M_SHARD // 128

    # ---- forward: merge per-core top-2 into global top-2 ----
    m1 = np.empty((N_CORES, n), np.float32)
    m2 = np.empty((N_CORES, n), np.float32)
    i1 = np.empty((N_CORES, n), np.int64)
    for c in range(N_CORES):
        vals = _decode_top8(results[c]["fwd_val"], n_chunks)
        idxs = _decode_top8(results[c]["fwd_idx"], n_chunks)
        m1[c] = vals[:, 0]
        m2[c] = vals[:, 1]
        i1[c] = idxs[:, 0].astype(np.int64) + c * M_SHARD

    w = np.argmax(m1, axis=0)                      # first max on ties
    rows = np.arange(n)
    s1 = m1[w, rows]
    fwd_nn = i1[w, rows]
    m1_masked = m1.copy()
    m1_masked[w, rows] = -np.inf
    s2 = np.maximum(m1_masked.max(axis=0), m2[w, rows]).astype(np.float32)

    # ---- backward: concatenate per-core full-column top-2 values ----
    cm1 = np.empty(M_KPTS, np.float32)
    cm2 = np.empty(M_KPTS, np.float32)
    for c in range(N_CORES):
        vals = _decode_top8(results[c]["bwd_val"], m_chunks)
        sl = slice(c * M_SHARD, (c + 1) * M_SHARD)
        cm1[sl] = vals[:, 0]
        cm2[sl] = vals[:, 1]

    # ---- exact reference arithmetic (float32) ----
    def dist(s):
        return SQRT_2 * np.sqrt(np.maximum(ONE - s, CLIP_LO))

    fd1, fd2 = dist(s1), dist(s2)
    fwd_ok = (fd1 / fd2) < np.float32(1.0)
    bd1, bd2 = dist(cm1), dist(cm2)
    bck_ok = (bd1 / bd2) < np.float32(1.0)

    # mutual NN: row i's best value must BE column j's max (bitwise; exact
    # because the backward path transposes the very same f32 tiles).  Ties
    # where this differs from index-equality are exactly the cases where
    # bck_ok / fwd_ok are False in both formulations.
    mutual = fwd_ok & bck_ok[fwd_nn] & (s1 == cm1[fwd_nn])

    indices0 = np.where(mutual, fwd_nn, -1)[None, :].astype(np.int32)
    mscores0 = (indices0 > 0).astype(np.int32)
    matches1 = np.full((1, M_KPTS), -1, dtype=np.int32)
    mscores1 = np.zeros((1, M_KPTS), dtype=np.float32)
    return indices0, matches1, mscores0, mscores1


def kernel(descriptors0, descriptors1, keypoints0, keypoints1):
    results = run_device(descriptors0, descriptors1)
    return postprocess(results)



# revision 23
# speedup vs baseline: 4.9784x; 4.9784x over previous
"""Trainium2 Bass kernel for Disk descriptor mutual-NN matching (retrieval_knn).

Strategy (8 NeuronCores, shard descriptors1 columns M across cores):
  The device computes, per core, a single compact CANDIDATE map; all exact
  arithmetic happens on the host over tiny candidate sets.

  Device pipeline per core (chunk pair j covers rows {256j..256j+255}):
    - fp8e4m3 DoubleRow matmuls (0.5 cyc/row): S chunks [128, 1024] in
      PSUM fp32.
    - chunk 2j:   ACT converts PSUM fp32 -> SBUF f16 (sp).
    - chunk 2j+1: DVE computes u_j = max(PSUM fp32 chunk, sp) -> f16, i.e.
      the elementwise pair-max over the two chunks, fusing the second
      conversion into the reduction (TensorTensor may read ONE PSUM input).
    - u maps stream to DRAM; no transposes, no top-k on device.

  u_j[p, c] = max(S[256j+p, c], S[256j+128+p, c]) serves BOTH directions:
    - forward:  row r's column scores are the map (r//256, r%128) row ->
      host top-32 columns cover the true top-2 (the sibling row dilutes
      ranks by ~lambda=4; 32 has ~15-sigma margin).
    - backward: column j's block scores over 4096 2-row blocks -> host
      top-16 blocks (32 rows).
  Host computes exact fp32 dots for the candidates only and applies the
  reference's exact ratio-test / mutual-NN arithmetic.
"""

import sys

if "/opt/trn_rl_repo" not in sys.path:
    sys.path.insert(0, "/opt/trn_rl_repo")

import numpy as np
import ml_dtypes

N_KPTS = 8192
M_KPTS = 8192
F_DIM = 256
N_CORES = 8
M_SHARD = M_KPTS // N_CORES      # 1024

N_CHUNKS = N_KPTS // 128         # 64 row chunks
N_PAIRS = N_CHUNKS // 2          # 32 chunk pairs (2-row blocks)

RBWD_W = N_PAIRS * M_SHARD       # 32768

FP8_SCALE = np.float32(8.0)

SQRT_2 = np.float32(1.414213)
CLIP_LO = np.float32(1e-6)
ONE = np.float32(1.0)

TOPC_FWD = 32                    # candidate columns per row
TOPB_BWD = 16                    # 2-row blocks per column (32 rows)

# pairs where BOTH conversions run on ACT and the pair-max runs f16 on DVE
# (load balancing: ACT is faster per element than DVE's fp32-rate read)
TYPE_B_PAIRS = frozenset((8, 24))


def build_kernel():
    import concourse.bacc as bacc
    import concourse.mybir as mybir
    import concourse.tile as tile

    nc = bacc.Bacc("TRN2", target_bir_lowering=False, debug=False,
                   num_devices=1)

    d0dr = nc.dram_tensor("d0dr", [128, 2, N_KPTS], mybir.dt.float8e4,
                          kind="ExternalInput")
    d1dr = nc.dram_tensor("d1dr", [128, 2, M_SHARD], mybir.dt.float8e4,
                          kind="ExternalInput")
    rbwd = nc.dram_tensor("rbwd", [128, RBWD_W], mybir.dt.float16,
                          kind="ExternalOutput")

    mx = mybir.AluOpType.max
    DR = mybir.MatmulPerfMode.DoubleRow

    with tile.TileContext(nc) as tc:
        with tc.tile_pool(name="persist", bufs=1) as persist, \
             tc.tile_pool(name="s16", bufs=6) as s16_pool, \
             tc.tile_pool(name="outs", bufs=1) as outs_pool, \
             tc.tile_pool(name="psf", bufs=4, space="PSUM") as psf:

            d0s = persist.tile([128, 2, N_KPTS], mybir.dt.float8e4,
                               name="d0s")
            d1s = persist.tile([128, 2, M_SHARD], mybir.dt.float8e4,
                               name="d1s")
            # small input first so the first matmul can start early
            nc.sync.dma_start(d1s[:, :, 0:512], d1dr[:, :, 0:512])
            nc.sync.dma_start(d1s[:, :, 512:1024], d1dr[:, :, 512:1024])
            # tiny first piece so the first matmul can start immediately
            bounds = [0, 256] + [1024 * i for i in range(1, 9)]
            for p in range(len(bounds) - 1):
                sl = slice(bounds[p], bounds[p + 1])
                nc.sync.dma_start(d0s[:, :, sl], d0dr[:, :, sl])

            u_out = outs_pool.tile([128, N_PAIRS, M_SHARD],
                                   mybir.dt.float16, name="u_out")

            def chunk_matmuls(n):
                pf = psf.tile([128, M_SHARD], mybir.dt.float32, tag="pf")
                for m in range(2):
                    nc.tensor.matmul(
                        pf[:, m * 512:(m + 1) * 512],
                        d0s[:, :, n * 128:(n + 1) * 128],
                        d1s[:, :, m * 512:(m + 1) * 512],
                        start=True, stop=True, perf_mode=DR)
                return pf

            for j in range(N_PAIRS):
                pf0 = chunk_matmuls(2 * j)
                sp0 = s16_pool.tile([128, M_SHARD], mybir.dt.float16,
                                    tag="sp")
                nc.scalar.copy(sp0[:], pf0[:])
                pf1 = chunk_matmuls(2 * j + 1)
                if j in TYPE_B_PAIRS:
                    sp1 = s16_pool.tile([128, M_SHARD], mybir.dt.float16,
                                        tag="sp")
                    nc.scalar.copy(sp1[:], pf1[:])
                    nc.vector.tensor_tensor(out=u_out[:, j, :], in0=sp0[:],
                                            in1=sp1[:], op=mx)
                else:
                    # fused: second conversion + pair-max in one DVE op
                    nc.vector.tensor_tensor(out=u_out[:, j, :], in0=pf1[:],
                                            in1=sp0[:], op=mx)
                # stream u out; finer slices near the end shorten the tail
                if j < 28:
                    flush = j % 4 == 3
                    lo = j - 3
                else:
                    flush = True
                    lo = j
                if flush:
                    sl = slice(lo * M_SHARD, (j + 1) * M_SHARD)
                    nc.sync.dma_start(
                        rbwd[:, sl],
                        u_out[:, lo:j + 1, :].rearrange(
                            "p a b -> p (a b)"))

    nc.compile()
    return nc


_KERNEL_CACHE = {}


def get_kernel():
    if "k" not in _KERNEL_CACHE:
        _KERNEL_CACHE["k"] = build_kernel()
    return _KERNEL_CACHE["k"]


# --------------------------------------------------------------------------
# Host side
# --------------------------------------------------------------------------

def make_core_inputs(d0, d1):
    """d0, d1: [256, 8192] float32 (full). Returns per-core input dicts."""
    d0_8 = (d0 * FP8_SCALE).astype(ml_dtypes.float8_e4m3fn)
    d1_8 = (d1 * FP8_SCALE).astype(ml_dtypes.float8_e4m3fn)
    # DoubleRow layout: [k, t, i] = x[t*128 + k, i]
    d0dr = np.ascontiguousarray(
        d0_8.reshape(2, 128, N_KPTS).transpose(1, 0, 2))
    in_maps = []
    for c in range(N_CORES):
        sh = d1_8[:, c * M_SHARD:(c + 1) * M_SHARD]
        d1dr = np.ascontiguousarray(
            sh.reshape(2, 128, M_SHARD).transpose(1, 0, 2))
        in_maps.append({"d0dr": d0dr, "d1dr": d1dr})
    return in_maps


def run_device(d0, d1):
    from concourse.bass_utils import run_bass_kernel_spmd

    nc = get_kernel()
    in_maps = make_core_inputs(d0, d1)
    last_err = None
    for _attempt in range(3):
        try:
            res = run_bass_kernel_spmd(nc, in_maps, list(range(N_CORES)))
            return res.results
        except Exception as e:  # rare transient device flakes
            last_err = e
    raise last_err


def _topk_idx(arr, k):
    """Indices of the k largest per row (unordered); torch is ~10x faster
    than np.argpartition on this host."""
    try:
        import torch
        return torch.topk(torch.from_numpy(arr), k, dim=1).indices.numpy()
    except ImportError:
        return np.argpartition(-arr, k - 1, axis=1)[:, :k]


def postprocess(results, d0, d1):
    """results: per-core {'rbwd'}; d0,d1 [256,8192] f32 full."""
    d0T = np.ascontiguousarray(d0.T)   # [N, F] f32
    d1T = np.ascontiguousarray(d1.T)   # [M, F] f32

    # u map: rb[core, p, j, c] = max(S[256j+p, core*1024+c],
    #                                S[256j+128+p, core*1024+c])
    rb = np.stack([r["rbwd"] for r in results])
    rb = rb.reshape(N_CORES, 128, N_PAIRS, M_SHARD)
    # bm[(j, p), global col] -- one map per 2-row block
    bm = np.ascontiguousarray(
        rb.transpose(2, 1, 0, 3).reshape(N_PAIRS * 128, M_KPTS)
    ).astype(np.float32)

    # ---- forward: rows r and r^128 share map (r//256, r%128) ----
    topc = _topk_idx(bm, TOPC_FWD)                       # [4096, K]
    r_all = np.arange(N_KPTS)
    map_id = (r_all // 256) * 128 + (r_all % 128)
    js = topc[map_id]                                   # [N, K] candidate cols

    s1 = np.empty(N_KPTS, np.float32)
    s2 = np.empty(N_KPTS, np.float32)
    fwd_nn = np.empty(N_KPTS, np.int64)
    slab = 2048
    for s in range(0, N_KPTS, slab):
        e = s + slab
        gath = d1T[js[s:e]]                                # [slab, K, F]
        dots = (gath * d0T[s:e, None, :]).sum(-1)          # [slab, K] f32
        m1 = dots.max(axis=1)
        nn = np.where(dots == m1[:, None], js[s:e], M_KPTS + 1).min(axis=1)
        mk = np.where(js[s:e] == nn[:, None], -np.inf, dots)
        s1[s:e] = m1
        s2[s:e] = mk.max(axis=1)
        fwd_nn[s:e] = nn

    # ---- backward: per column, top blocks over the 4096 2-row blocks ----
    bmT = np.ascontiguousarray(bm.T)                       # [M, 4096]
    topb = _topk_idx(bmT, TOPB_BWD)
    jj, pp = np.divmod(topb, 128)
    rows = np.stack([jj * 256 + pp, jj * 256 + 128 + pp],
                    axis=2).reshape(M_KPTS, -1)            # [M, 2*TOPB]

    cm1 = np.empty(M_KPTS, np.float32)
    cm2 = np.empty(M_KPTS, np.float32)
    bck_nn = np.empty(M_KPTS, np.int64)
    for s in range(0, M_KPTS, slab):
        e = s + slab
        g2 = d0T[rows[s:e]]                          # [slab, 2*TOPB, F]
        dd = (g2 * d1T[s:e, None, :]).sum(-1)        # [slab, 2*TOPB] f32
        m1 = dd.max(axis=1)
        nn = np.where(dd == m1[:, None], rows[s:e], N_KPTS + 1).min(axis=1)
        mk = np.where(rows[s:e] == nn[:, None], -np.inf, dd)
        cm1[s:e] = m1
        cm2[s:e] = mk.max(axis=1)
        bck_nn[s:e] = nn

    # ---- exact reference arithmetic (float32) ----
    def dist(s):
        return SQRT_2 * np.sqrt(np.maximum(ONE - s.astype(np.float32),
                                           CLIP_LO))

    fwd_ok = (dist(s1) / dist(s2)) < ONE
    bck_ok = (dist(cm1) / dist(cm2)) < ONE

    mutual = fwd_ok & bck_ok[fwd_nn] & (bck_nn[fwd_nn] == np.arange(N_KPTS))

    indices0 = np.where(mutual, fwd_nn, -1)[None, :].astype(np.int32)
    mscores0 = (indices0 > 0).astype(np.int32)
    matches1 = np.full((1, M_KPTS), -1, dtype=np.int32)
    mscores1 = np.zeros((1, M_KPTS), dtype=np.float32)
    return indices0, matches1, mscores0, mscores1


def kernel(descriptors0, descriptors1, keypoints0, keypoints1):
    d0 = np.ascontiguousarray(descriptors0[0]).astype(np.float32, copy=False)
    d1 = np.ascontiguousarray(descriptors1[0]).astype(np.float32, copy=False)
    results = run_device(d0, d1)
    return postprocess(results, d0, d1)


# revision 27
# speedup vs baseline: 5.0598x; 1.0163x over previous
"""Trainium2 Bass kernel for Disk descriptor mutual-NN matching (retrieval_knn).

Strategy (8 NeuronCores, shard descriptors1 columns M across cores):
  The device computes, per core, a single compact CANDIDATE map; all exact
  arithmetic happens on the host over tiny candidate sets.

  Device pipeline per core (chunk pair j covers rows {256j..256j+255}):
    - fp8e4m3 DoubleRow matmuls (0.5 cyc/row): S chunks [128, 1024] in
      PSUM fp32.
    - chunk 2j:   ACT converts PSUM fp32 -> SBUF f16 (sp).
    - chunk 2j+1: DVE computes u_j = max(PSUM fp32 chunk, sp) -> f16, i.e.
      the elementwise pair-max over the two chunks, fusing the second
      conversion into the reduction (TensorTensor may read ONE PSUM input).
    - u maps stream to DRAM; no transposes, no top-k on device.

  u_j[p, c] = max(S[256j+p, c], S[256j+128+p, c]) serves BOTH directions:
    - forward:  row r's column scores are the map (r//256, r%128) row ->
      host top-32 columns cover the true top-2 (the sibling row dilutes
      ranks by ~lambda=4; 32 has ~15-sigma margin).
    - backward: column j's block scores over 4096 2-row blocks -> host
      top-16 blocks (32 rows).
  Host computes exact fp32 dots for the candidates only and applies the
  reference's exact ratio-test / mutual-NN arithmetic.
"""

import sys

if "/opt/trn_rl_repo" not in sys.path:
    sys.path.insert(0, "/opt/trn_rl_repo")

import numpy as np
import ml_dtypes

N_KPTS = 8192
M_KPTS = 8192
F_DIM = 256
N_CORES = 8
M_SHARD = M_KPTS // N_CORES      # 1024

N_CHUNKS = N_KPTS // 128         # 64 row chunks
N_PAIRS = N_CHUNKS // 2          # 32 chunk pairs (2-row blocks)

RBWD_W = N_PAIRS * M_SHARD       # 32768

FP8_SCALE = np.float32(8.0)

SQRT_2 = np.float32(1.414213)
CLIP_LO = np.float32(1e-6)
ONE = np.float32(1.0)

TOPC_FWD = 32                    # candidate columns per row
TOPB_BWD = 16                    # 2-row blocks per column (32 rows)

# pairs where BOTH conversions run on ACT and the pair-max runs f16 on DVE
# (load balancing: ACT is faster per element than DVE's fp32-rate read)
TYPE_B_PAIRS = frozenset((5, 16, 27))


def build_kernel():
    import concourse.bacc as bacc
    import concourse.mybir as mybir
    import concourse.tile as tile

    nc = bacc.Bacc("TRN2", target_bir_lowering=False, debug=False,
                   num_devices=1)

    d0dr = nc.dram_tensor("d0dr", [128, 2, N_KPTS], mybir.dt.float8e4,
                          kind="ExternalInput")
    d1dr = nc.dram_tensor("d1dr", [128, 2, M_SHARD], mybir.dt.float8e4,
                          kind="ExternalInput")
    rbwd = nc.dram_tensor("rbwd", [128, RBWD_W], mybir.dt.float16,
                          kind="ExternalOutput")

    mx = mybir.AluOpType.max
    DR = mybir.MatmulPerfMode.DoubleRow

    with tile.TileContext(nc) as tc:
        with tc.tile_pool(name="persist", bufs=1) as persist, \
             tc.tile_pool(name="s16", bufs=6) as s16_pool, \
             tc.tile_pool(name="outs", bufs=1) as outs_pool, \
             tc.tile_pool(name="psf", bufs=4, space="PSUM") as psf:

            d0s = persist.tile([128, 2, N_KPTS], mybir.dt.float8e4,
                               name="d0s")
            d1s = persist.tile([128, 2, M_SHARD], mybir.dt.float8e4,
                               name="d1s")
            # tiny first pieces so the first matmul can start immediately
            nc.sync.dma_start(d0s[:, :, 0:256], d0dr[:, :, 0:256])
            nc.sync.dma_start(d1s[:, :, 0:512], d1dr[:, :, 0:512])
            nc.sync.dma_start(d1s[:, :, 512:1024], d1dr[:, :, 512:1024])
            bounds = [256, 2048, 4096, 6144, 8192]
            for p in range(len(bounds) - 1):
                sl = slice(bounds[p], bounds[p + 1])
                nc.sync.dma_start(d0s[:, :, sl], d0dr[:, :, sl])

            u_out = outs_pool.tile([128, N_PAIRS, M_SHARD],
                                   mybir.dt.float16, name="u_out")

            def chunk_matmuls(n):
                pf = psf.tile([128, M_SHARD], mybir.dt.float32, tag="pf")
                for m in range(2):
                    nc.tensor.matmul(
                        pf[:, m * 512:(m + 1) * 512],
                        d0s[:, :, n * 128:(n + 1) * 128],
                        d1s[:, :, m * 512:(m + 1) * 512],
                        start=True, stop=True, perf_mode=DR)
                return pf

            for j in range(N_PAIRS):
                pf0 = chunk_matmuls(2 * j)
                sp0 = s16_pool.tile([128, M_SHARD], mybir.dt.float16,
                                    tag="sp")
                nc.scalar.copy(sp0[:], pf0[:])
                pf1 = chunk_matmuls(2 * j + 1)
                if j in TYPE_B_PAIRS:
                    sp1 = s16_pool.tile([128, M_SHARD], mybir.dt.float16,
                                        tag="sp")
                    nc.scalar.copy(sp1[:], pf1[:])
                    nc.vector.tensor_tensor(out=u_out[:, j, :], in0=sp0[:],
                                            in1=sp1[:], op=mx)
                elif j == N_PAIRS - 1:
                    # split the last fused op so its first half's DMA
                    # overlaps the second half (shorter tail)
                    for hh in range(2):
                        sl2 = slice(hh * 512, (hh + 1) * 512)
                        nc.vector.tensor_tensor(out=u_out[:, j, sl2],
                                                in0=pf1[:, sl2],
                                                in1=sp0[:, sl2], op=mx)
                        nc.sync.dma_start(
                            rbwd[:, j * M_SHARD + hh * 512:
                                 j * M_SHARD + (hh + 1) * 512],
                            u_out[:, j, sl2])
                    continue
                else:
                    # fused: second conversion + pair-max in one DVE op
                    nc.vector.tensor_tensor(out=u_out[:, j, :], in0=pf1[:],
                                            in1=sp0[:], op=mx)
                # stream u out; finer slices near the end shorten the tail
                if j < 28:
                    flush = j % 4 == 3
                    lo = j - 3
                else:
                    flush = True
                    lo = j
                if flush:
                    sl = slice(lo * M_SHARD, (j + 1) * M_SHARD)
                    nc.sync.dma_start(
                        rbwd[:, sl],
                        u_out[:, lo:j + 1, :].rearrange(
                            "p a b -> p (a b)"))

    nc.compile()
    return nc


_KERNEL_CACHE = {}


def get_kernel():
    if "k" not in _KERNEL_CACHE:
        _KERNEL_CACHE["k"] = build_kernel()
    return _KERNEL_CACHE["k"]


# --------------------------------------------------------------------------
# Host side
# --------------------------------------------------------------------------

def make_core_inputs(d0, d1):
    """d0, d1: [256, 8192] float32 (full). Returns per-core input dicts."""
    d0_8 = (d0 * FP8_SCALE).astype(ml_dtypes.float8_e4m3fn)
    d1_8 = (d1 * FP8_SCALE).astype(ml_dtypes.float8_e4m3fn)
    # DoubleRow layout: [k, t, i] = x[t*128 + k, i]
    d0dr = np.ascontiguousarray(
        d0_8.reshape(2, 128, N_KPTS).transpose(1, 0, 2))
    in_maps = []
    for c in range(N_CORES):
        sh = d1_8[:, c * M_SHARD:(c + 1) * M_SHARD]
        d1dr = np.ascontiguousarray(
            sh.reshape(2, 128, M_SHARD).transpose(1, 0, 2))
        in_maps.append({"d0dr": d0dr, "d1dr": d1dr})
    return in_maps


def run_device(d0, d1):
    from concourse.bass_utils import run_bass_kernel_spmd

    nc = get_kernel()
    in_maps = make_core_inputs(d0, d1)
    last_err = None
    for _attempt in range(3):
        try:
            res = run_bass_kernel_spmd(nc, in_maps, list(range(N_CORES)))
            return res.results
        except Exception as e:  # rare transient device flakes
            last_err = e
    raise last_err


def _topk_idx(arr, k):
    """Indices of the k largest per row (unordered); torch is ~10x faster
    than np.argpartition on this host."""
    try:
        import torch
        return torch.topk(torch.from_numpy(arr), k, dim=1).indices.numpy()
    except ImportError:
        return np.argpartition(-arr, k - 1, axis=1)[:, :k]


def postprocess(results, d0, d1):
    """results: per-core {'rbwd'}; d0,d1 [256,8192] f32 full."""
    d0T = np.ascontiguousarray(d0.T)   # [N, F] f32
    d1T = np.ascontiguousarray(d1.T)   # [M, F] f32

    # u map: rb[core, p, j, c] = max(S[256j+p, core*1024+c],
    #                                S[256j+128+p, core*1024+c])
    rb = np.stack([r["rbwd"] for r in results])
    rb = rb.reshape(N_CORES, 128, N_PAIRS, M_SHARD)
    # bm[(j, p), global col] -- one map per 2-row block
    bm = np.ascontiguousarray(
        rb.transpose(2, 1, 0, 3).reshape(N_PAIRS * 128, M_KPTS)
    ).astype(np.float32)

    # ---- forward: rows r and r^128 share map (r//256, r%128) ----
    topc = _topk_idx(bm, TOPC_FWD)                       # [4096, K]
    r_all = np.arange(N_KPTS)
    map_id = (r_all // 256) * 128 + (r_all % 128)
    js = topc[map_id]                                   # [N, K] candidate cols

    s1 = np.empty(N_KPTS, np.float32)
    s2 = np.empty(N_KPTS, np.float32)
    fwd_nn = np.empty(N_KPTS, np.int64)
    slab = 2048
    for s in range(0, N_KPTS, slab):
        e = s + slab
        gath = d1T[js[s:e]]                                # [slab, K, F]
        dots = (gath * d0T[s:e, None, :]).sum(-1)          # [slab, K] f32
        m1 = dots.max(axis=1)
        nn = np.where(dots == m1[:, None], js[s:e], M_KPTS + 1).min(axis=1)
        mk = np.where(js[s:e] == nn[:, None], -np.inf, dots)
        s1[s:e] = m1
        s2[s:e] = mk.max(axis=1)
        fwd_nn[s:e] = nn

    # ---- backward: per column, top blocks over the 4096 2-row blocks ----
    bmT = np.ascontiguousarray(bm.T)                       # [M, 4096]
    topb = _topk_idx(bmT, TOPB_BWD)
    jj, pp = np.divmod(topb, 128)
    rows = np.stack([jj * 256 + pp, jj * 256 + 128 + pp],
                    axis=2).reshape(M_KPTS, -1)            # [M, 2*TOPB]

    cm1 = np.empty(M_KPTS, np.float32)
    cm2 = np.empty(M_KPTS, np.float32)
    bck_nn = np.empty(M_KPTS, np.int64)
    for s in range(0, M_KPTS, slab):
        e = s + slab
        g2 = d0T[rows[s:e]]                          # [slab, 2*TOPB, F]
        dd = (g2 * d1T[s:e, None, :]).sum(-1)        # [slab, 2*TOPB] f32
        m1 = dd.max(axis=1)
        nn = np.where(dd == m1[:, None], rows[s:e], N_KPTS + 1).min(axis=1)
        mk = np.where(rows[s:e] == nn[:, None], -np.inf, dd)
        cm1[s:e] = m1
        cm2[s:e] = mk.max(axis=1)
        bck_nn[s:e] = nn

    # ---- exact reference arithmetic (float32) ----
    def dist(s):
        return SQRT_2 * np.sqrt(np.maximum(ONE - s.astype(np.float32),
                                           CLIP_LO))

    fwd_ok = (dist(s1) / dist(s2)) < ONE
    bck_ok = (dist(cm1) / dist(cm2)) < ONE

    mutual = fwd_ok & bck_ok[fwd_nn] & (bck_nn[fwd_nn] == np.arange(N_KPTS))

    indices0 = np.where(mutual, fwd_nn, -1)[None, :].astype(np.int32)
    mscores0 = (indices0 > 0).astype(np.int32)
    matches1 = np.full((1, M_KPTS), -1, dtype=np.int32)
    mscores1 = np.zeros((1, M_KPTS), dtype=np.float32)
    return indices0, matches1, mscores0, mscores1


def kernel(descriptors0, descriptors1, keypoints0, keypoints1):
    d0 = np.ascontiguousarray(descriptors0[0]).astype(np.float32, copy=False)
    d1 = np.ascontiguousarray(descriptors1[0]).astype(np.float32, copy=False)
    results = run_device(d0, d1)
    return postprocess(results, d0, d1)
